# revision 46
# baseline (speedup 1.0000x reference)
"""MoE (top-2 of 8 experts) Trainium2 kernel — expert-parallel across 8 cores.

Strategy (hardcoded for B,S,H,I,E = 1,2048,2048,8192,8; T=2048; top-2):
  - Host (numpy, exact fp64 gate): logits -> softmax -> top-2 -> per-expert
    token lists + combine weights g = score * alpha[e]. Host gathers each
    expert's C=max-load tokens, lays out all matmul operands in DoubleRow
    [k/256, 2, n] order and splits every operand X into an fp8e4 residual
    pair (Xh = fp8(X*s), Xl = fp8(X*s - Xh); power-of-2 scales absorbed in
    the gelu input scale and the output g vector).
  - Device, core e (PE-roofline bound, fp8 DoubleRow at 0.5 cyc/row =
    4x fp16 per unit contraction; 3-term products Wh*Xh + Wh*Xl + Wl*Xh
    give ~fp16-level accuracy at 0.75x fp16 PE time):
      fc1: h[i, c] = gelu((w1^T xg)/(SA*SB) + b1), split on-chip into the
           fp8 pair (hh, hl) scaled by SC        (w1 stationary, x moving)
      fc2: yT[h, c] = (w2^T h) * g[c]/(SA*SC)    (w2 stationary, h moving)
    The first w1 group runs its 6 CA-chains ks-outer-interleaved so early
    PE work tracks the DMA arrival frontier; hi+lo weight streams ride the
    SP HWDGE queue; x rides ACT; capacity overflow columns (C-512) run as
    trailing mini-chains on 2 shared PSUM banks.
  - Host combine: out[tok_e] += yT_e.T rows; the (gates @ fc2_b) bias term
    is added on host. Output fp32. rel err ~1.9e-3 vs fp32 reference.
"""

import numpy as np

# ---- problem constants ----
B, S_SEQ, H, I, E = 1, 2048, 2048, 8192, 8
T = B * S_SEQ
P = 128
HT = H // P          # 16 h-tiles
IT = I // P          # 64 i-tiles
TOP_K = 2

_COMPILED = {}

# fp8 pipeline scales (powers of 2): weights *SA, x *SB, h *SC
SA = 256.0
SB = 16.0
SC = 32.0


def _build(C):
    """fp16 expert-MLP kernel with capacity C (<= 1024)."""
    import concourse.mybir as mybir
    import concourse.tile as tile
    from concourse import bacc

    dt = mybir.dt
    AF = mybir.ActivationFunctionType
    OP = mybir.AluOpType

    CA = min(C, 512)
    CB = C - CA
    assert 0 < C <= 1024

    nc = bacc.Bacc("TRN2", target_bir_lowering=False, num_devices=8)

    w1_d = nc.dram_tensor("w1t", [H, I], dt.float16, kind="ExternalInput")
    w2_d = nc.dram_tensor("w2", [I, H], dt.float16, kind="ExternalInput")
    xgt_d = nc.dram_tensor("xgt", [H, C], dt.float16, kind="ExternalInput")
    g_d = nc.dram_tensor("g", [P, C], dt.float32, kind="ExternalInput")
    b1_d = nc.dram_tensor("b1", [P, IT], dt.float32, kind="ExternalInput")
    yt_d = nc.dram_tensor("yt", [H, C], dt.float16, kind="ExternalOutput")

    G8 = 8           # w1 i-col groups of 1024 (8 i-tiles each)

    with tile.TileContext(nc) as tc:
        with tc.tile_pool(name="pers", bufs=1) as pers:
            b1_sb = pers.tile([P, IT], dt.float32, tag="b1", name="b1_sb")
            nc.gpsimd.dma_start(b1_sb[:], b1_d[:])
            g_sb = pers.tile([P, C], dt.float32, tag="g", name="g_sb")
            nc.gpsimd.dma_start(g_sb[:], g_d[:])
            # xgT k-tiles on the ACT queue (SP is busy with w1)
            xgT = [pers.tile([P, C], dt.float16, tag=f"xgT{k}", name=f"xgT{k}")
                   for k in range(HT)]
            for k in range(HT):
                nc.scalar.dma_start(xgT[k][:], xgt_d[k * P:(k + 1) * P, :])
            h1 = [pers.tile([P, C], dt.float16, tag=f"h1_{it}", name=f"h1_{it}")
                  for it in range(IT)]

            # ---- fc1: w1 stationary, xgT moving ----
            w1p = tc.tile_pool(name="w1p", bufs=24)
            w1pool = w1p.__enter__()
            w1_tiles = {}

            def fetch_w1(g8):
                for k in range(HT):
                    t = w1pool.tile([P, 1024], dt.float16, tag="w1", name="w1")
                    nc.sync.dma_start(
                        t[:], w1_d[k * P:(k + 1) * P, g8 * 1024:(g8 + 1) * 1024])
                    w1_tiles[(g8, k)] = t

            fetch_w1(0)
            with (
                tc.tile_pool(name="ps1a", bufs=3, space="PSUM") as ps1a,
                tc.tile_pool(name="ps1b", bufs=3, space="PSUM") as ps1b,
            ):
                for g8 in range(G8):
                    if g8 + 1 < G8:
                        fetch_w1(g8 + 1)
                    for it8 in range(8):
                        it = g8 * 8 + it8
                        pa = ps1a.tile([P, CA], dt.float32, tag="pa", name="pa")
                        pb = ps1b.tile([P, CB], dt.float32, tag="pb", name="pb") if CB else None
                        for k in range(HT):
                            lhsT = w1_tiles[(g8, k)][:, it8 * P:(it8 + 1) * P]
                            nc.tensor.matmul(pa[:], lhsT, xgT[k][:, 0:CA],
                                             start=(k == 0), stop=(k == HT - 1))
                            if CB:
                                nc.tensor.matmul(pb[:], lhsT, xgT[k][:, CA:C],
                                                 start=(k == 0), stop=(k == HT - 1))
                        bias = b1_sb[:, it:it + 1]
                        nc.scalar.activation(h1[it][:, 0:CA], pa[:],
                                             AF.Gelu_apprx_tanh, bias=bias)
                        if CB:
                            nc.scalar.activation(h1[it][:, CA:C], pb[:],
                                                 AF.Gelu_apprx_tanh, bias=bias)
                    for k in range(HT):
                        del w1_tiles[(g8, k)]
            w1p.__exit__(None, None, None)

            # ---- fc2: w2 stationary, h1 moving, out yT[h, c] ----
            with (
                tc.tile_pool(name="w2p", bufs=32) as w2pool,
                tc.tile_pool(name="ps2a", bufs=1, space="PSUM") as ps2a,
                tc.tile_pool(name="ps2b", bufs=1, space="PSUM") as ps2b,
                tc.tile_pool(name="ytp", bufs=4) as ytp,
            ):
                w2_tiles = {}

                def fetch_w2(hg):
                    for i in range(IT):
                        t = w2pool.tile([P, 512], dt.float16, tag="w2", name="w2")
                        nc.sync.dma_start(
                            t[:], w2_d[i * P:(i + 1) * P, hg * 512:(hg + 1) * 512])
                        w2_tiles[(hg, i)] = t

                fetch_w2(0)
                for hg in range(4):
                    if hg + 1 < 4:
                        fetch_w2(hg + 1)
                    pas = [ps2a.tile([P, CA], dt.float32, tag=f"fa{ht}",
                                     name=f"fa{ht}") for ht in range(4)]
                    pbs = ([ps2b.tile([P, CB], dt.float32, tag=f"fb{ht}",
                                      name=f"fb{ht}") for ht in range(4)]
                           if CB else None)
                    for i in range(IT):
                        w2t = w2_tiles[(hg, i)]
                        for ht in range(4):
                            lhsT = w2t[:, ht * P:(ht + 1) * P]
                            nc.tensor.matmul(pas[ht][:], lhsT, h1[i][:, 0:CA],
                                             start=(i == 0), stop=(i == IT - 1))
                            if CB:
                                nc.tensor.matmul(pbs[ht][:], lhsT, h1[i][:, CA:C],
                                                 start=(i == 0), stop=(i == IT - 1))
                    for ht in range(4):
                        hrow = hg * 4 + ht
                        yt = ytp.tile([P, C], dt.float16, tag="yt", name="yt")
                        nc.vector.tensor_tensor(out=yt[:, 0:CA], in0=pas[ht][:],
                                                in1=g_sb[:, 0:CA], op=OP.mult)
                        if CB:
                            nc.vector.tensor_tensor(out=yt[:, CA:C], in0=pbs[ht][:],
                                                    in1=g_sb[:, CA:C], op=OP.mult)
                        nc.scalar.dma_start(yt_d[hrow * P:(hrow + 1) * P, :], yt[:])
                    for i in range(IT):
                        del w2_tiles[(hg, i)]

    nc.compile()
    return nc


def _build_fp8(C):
    """fp8e4 DoubleRow 3-term residual kernel with capacity C (<= 1024).

    Each matmul operand X is split as Xh = fp8(X*s), Xl = fp8(X*s - Xh);
    products accumulate Wh*Xh + Wh*Xl + Wl*Xh in one PSUM group (shared
    power-of-2 scale, undone in the gelu input scale / output g scale).
    DoubleRow packs k=256 per matmul at 0.5 cyc/row -> 0.75x fp16 PE time.
    """
    import concourse.mybir as mybir
    import concourse.tile as tile
    from concourse import bacc

    dt = mybir.dt
    AF = mybir.ActivationFunctionType
    OP = mybir.AluOpType
    DR = mybir.MatmulPerfMode.DoubleRow

    CA = min(C, 512)
    CB = C - CA
    assert 0 < C <= 1024
    KS1 = H // 256       # 8 DR k-steps in fc1
    KS2 = I // 256       # 32 DR k-steps in fc2
    IP = IT // 2         # 32 h1 i-pairs

    nc = bacc.Bacc("TRN2", target_bir_lowering=False, num_devices=8)

    w1h_d = nc.dram_tensor("w1h", [KS1 * P, 2, I], dt.float8e4, kind="ExternalInput")
    w1l_d = nc.dram_tensor("w1l", [KS1 * P, 2, I], dt.float8e4, kind="ExternalInput")
    w2h_d = nc.dram_tensor("w2h", [KS2 * P, 2, H], dt.float8e4, kind="ExternalInput")
    w2l_d = nc.dram_tensor("w2l", [KS2 * P, 2, H], dt.float8e4, kind="ExternalInput")
    xhl_d = nc.dram_tensor("xhl", [KS1 * P, 2, 2 * C], dt.float8e4,
                           kind="ExternalInput")
    g_d = nc.dram_tensor("g", [P, C], dt.float32, kind="ExternalInput")
    b1_d = nc.dram_tensor("b1", [P, IT], dt.float32, kind="ExternalInput")
    yt_d = nc.dram_tensor("yt", [H, C], dt.float16, kind="ExternalOutput")

    G8 = 8               # w1 i-col groups of 1024 (8 i-tiles each)

    with tile.TileContext(nc) as tc:
        with tc.tile_pool(name="pers", bufs=1) as pers:
            xhl = [pers.tile([P, 2, 2 * C], dt.float8e4, tag=f"xhl{k}",
                             name=f"xhl{k}") for k in range(KS1)]
            for k in range(KS1):
                nc.scalar.dma_start(xhl[k][:], xhl_d[k * P:(k + 1) * P, :, :])

            b1_sb = pers.tile([P, IT], dt.float32, tag="b1", name="b1_sb")
            nc.gpsimd.dma_start(b1_sb[:], b1_d[:])
            g_sb = pers.tile([P, C], dt.float32, tag="g", name="g_sb")
            nc.gpsimd.dma_start(g_sb[:], g_d[:])
            hh = [pers.tile([P, 2, C], dt.float8e4, tag=f"hh{ip}", name=f"hh{ip}")
                  for ip in range(IP)]
            hl = [pers.tile([P, 2, C], dt.float8e4, tag=f"hl{ip}", name=f"hl{ip}")
                  for ip in range(IP)]

            # ---- fc1 ----
            # w2p opens first so it owns a disjoint SBUF region: its DMAs
            # prefetch during fc1 with no address-reuse deps on w1 tiles.
            w2p = tc.tile_pool(name="w2p", bufs=23)
            w2pool = w2p.__enter__()
            w2_tiles = {}

            def fetch_w2(hg):
                for i2 in range(I // 256):
                    th2 = w2pool.tile([P, 2, 512], dt.float8e4, tag="w2h",
                                      name="w2ht")
                    nc.sync.dma_start(
                        th2[:], w2h_d[i2 * P:(i2 + 1) * P, :,
                                      hg * 512:(hg + 1) * 512])
                    tl2 = w2pool.tile([P, 2, 512], dt.float8e4, tag="w2l",
                                      name="w2lt")
                    nc.sync.dma_start(
                        tl2[:], w2l_d[i2 * P:(i2 + 1) * P, :,
                                      hg * 512:(hg + 1) * 512])
                    w2_tiles[(hg, i2)] = (th2, tl2)

            w1p = tc.tile_pool(name="w1p", bufs=16)
            w1pool = w1p.__enter__()
            w1_tiles = {}

            def fetch_w1(g8):
                for ks in range(KS1):
                    th = w1pool.tile([P, 2, 1024], dt.float8e4, tag="w1h", name="w1ht")
                    nc.sync.dma_start(
                        th[:], w1h_d[ks * P:(ks + 1) * P, :,
                                     g8 * 1024:(g8 + 1) * 1024])
                    tl = w1pool.tile([P, 2, 1024], dt.float8e4, tag="w1l", name="w1lt")
                    nc.sync.dma_start(
                        tl[:], w1l_d[ks * P:(ks + 1) * P, :,
                                     g8 * 1024:(g8 + 1) * 1024])
                    w1_tiles[(g8, ks)] = (th, tl)

            fetch_w1(0)
            NKO = 6          # chains interleaved ks-outer (6 CA banks + 2 CB)
            KS_OUTER_G8 = 1  # groups using the ks-outer fill schedule
            TERMS = ((True, False), (True, True), (False, False))  # (hi_w, use_xl)
            with (
                tc.tile_pool(name="ps1a", bufs=NKO, space="PSUM") as ps1a,
                tc.tile_pool(name="ps1b", bufs=2, space="PSUM") as ps1b,
                tc.tile_pool(name="hring", bufs=3) as hring,
            ):
                n_t = 3 * KS1
                for g8 in range(G8):
                    if g8 + 1 < G8:
                        fetch_w1(g8 + 1)
                    if g8 >= KS_OUTER_G8:
                        # fully prefetched: paired CA+CB chains, drains overlap
                        for it8 in range(8):
                            it = g8 * 8 + it8
                            ip, sub = it // 2, it % 2
                            pa = ps1a.tile([P, CA], dt.float32, tag="pa",
                                           name="pa")
                            pb = (ps1b.tile([P, CB], dt.float32, tag="pb",
                                            name="pb")[:] if CB else None)
                            isl = slice(it8 * P, (it8 + 1) * P)
                            ti = 0
                            for ks in range(KS1):
                                th, tl = w1_tiles[(g8, ks)]
                                for hi_w, use_xl in TERMS:
                                    wt = th if hi_w else tl
                                    xo = C if use_xl else 0
                                    nc.tensor.matmul(
                                        pa[:], wt[:, :, isl],
                                        xhl[ks][:, :, xo:xo + CA],
                                        start=(ti == 0), stop=(ti == n_t - 1),
                                        perf_mode=DR)
                                    if CB:
                                        nc.tensor.matmul(
                                            pb, wt[:, :, isl],
                                            xhl[ks][:, :, xo + CA:xo + C],
                                            start=(ti == 0), stop=(ti == n_t - 1),
                                            perf_mode=DR)
                                    ti += 1
                            h16 = hring.tile([P, C], dt.float16, tag="h16",
                                             name="h16")
                            bias = b1_sb[:, it:it + 1]
                            nc.scalar.activation(h16[:, 0:CA], pa[:],
                                                 AF.Gelu_apprx_tanh, bias=bias,
                                                 scale=1.0 / (SA * SB))
                            if CB:
                                nc.scalar.activation(h16[:, CA:C], pb,
                                                     AF.Gelu_apprx_tanh,
                                                     bias=bias,
                                                     scale=1.0 / (SA * SB))
                            t16 = hring.tile([P, C], dt.float16, tag="t16",
                                             name="t16")
                            nc.scalar.activation(t16[:], h16[:], AF.Copy,
                                                 bias=0.0, scale=SC)
                            nc.vector.tensor_copy(hh[ip][:, sub, :], t16[:])
                            nc.vector.tensor_tensor(out=hl[ip][:, sub, :],
                                                    in0=t16[:],
                                                    in1=hh[ip][:, sub, :],
                                                    op=OP.subtract)
                        for ks in range(KS1):
                            del w1_tiles[(g8, ks)]
                        if g8 == G8 - 2:
                            fetch_w2(0)
                        elif g8 == G8 - 1:
                            fetch_w2(1)
                        continue
                    pas = [ps1a.tile([P, CA], dt.float32, tag="pa", name="pa")
                           for _ in range(NKO)]
                    h16s, geludone = {}, set()

                    def gelu_ca(it8, g8=g8):
                        it = g8 * 8 + it8
                        h16 = hring.tile([P, C], dt.float16, tag="h16", name="h16")
                        nc.scalar.activation(h16[:, 0:CA], pas[it8][:],
                                             AF.Gelu_apprx_tanh,
                                             bias=b1_sb[:, it:it + 1],
                                             scale=1.0 / (SA * SB))
                        h16s[it8] = h16
                        geludone.add(it8)

                    # interleave the first NKO it-chains ks-outer so early PE
                    # work tracks the w1/x DMA arrival frontier tile by tile
                    for ks in range(KS1):
                        th, tl = w1_tiles[(g8, ks)]
                        for ti3, (hi_w, use_xl) in enumerate(TERMS):
                            wt = th if hi_w else tl
                            xo = C if use_xl else 0
                            for it8 in range(NKO):
                                isl = slice(it8 * P, (it8 + 1) * P)
                                nc.tensor.matmul(
                                    pas[it8][:], wt[:, :, isl],
                                    xhl[ks][:, :, xo:xo + CA],
                                    start=(ks == 0 and ti3 == 0),
                                    stop=(ks == KS1 - 1 and ti3 == len(TERMS) - 1),
                                    perf_mode=DR)
                    # remaining chains term-inner (banks free as gelus drain)
                    for it8 in range(NKO, 8):
                        gelu_ca(it8 - NKO)
                        pa = ps1a.tile([P, CA], dt.float32, tag="pa", name="pa")
                        pas.append(pa)
                        isl = slice(it8 * P, (it8 + 1) * P)
                        ti = 0
                        for ks in range(KS1):
                            th, tl = w1_tiles[(g8, ks)]
                            for hi_w, use_xl in TERMS:
                                wt = th if hi_w else tl
                                xo = C if use_xl else 0
                                nc.tensor.matmul(
                                    pa[:], wt[:, :, isl],
                                    xhl[ks][:, :, xo:xo + CA],
                                    start=(ti == 0), stop=(ti == n_t - 1),
                                    perf_mode=DR)
                                ti += 1
                    # CB overflow mini-chains (slots 512..C) + per-it split
                    for it8 in range(8):
                        it = g8 * 8 + it8
                        ip, sub = it // 2, it % 2
                        if it8 not in geludone:
                            gelu_ca(it8)
                        h16 = h16s[it8]
                        if CB:
                            pb = ps1b.tile([P, CB], dt.float32, tag="pb",
                                           name="pb")[:]
                            ti = 0
                            for ks in range(KS1):
                                th, tl = w1_tiles[(g8, ks)]
                                for hi_w, use_xl in TERMS:
                                    wt = th if hi_w else tl
                                    xo = C if use_xl else 0
                                    nc.tensor.matmul(
                                        pb, wt[:, :, it8 * P:(it8 + 1) * P],
                                        xhl[ks][:, :, xo + CA:xo + C],
                                        start=(ti == 0), stop=(ti == n_t - 1),
                                        perf_mode=DR)
                                    ti += 1
                            nc.scalar.activation(h16[:, CA:C], pb,
                                                 AF.Gelu_apprx_tanh,
                                                 bias=b1_sb[:, it:it + 1],
                                                 scale=1.0 / (SA * SB))
                        t16 = hring.tile([P, C], dt.float16, tag="t16", name="t16")
                        nc.scalar.activation(t16[:], h16[:], AF.Copy,
                                             bias=0.0, scale=SC)
                        nc.vector.tensor_copy(hh[ip][:, sub, :], t16[:])
                        nc.vector.tensor_tensor(out=hl[ip][:, sub, :],
                                                in0=t16[:], in1=hh[ip][:, sub, :],
                                                op=OP.subtract)
                    for ks in range(KS1):
                        del w1_tiles[(g8, ks)]
                    if g8 == G8 - 2:
                        fetch_w2(0)
                    elif g8 == G8 - 1:
                        fetch_w2(1)
            w1p.__exit__(None, None, None)

            # ---- fc2 ----
            with (
                tc.tile_pool(name="ps2a", bufs=1, space="PSUM") as ps2a,
                tc.tile_pool(name="ps2b", bufs=1, space="PSUM") as ps2b,
                tc.tile_pool(name="ytp", bufs=4) as ytp,
            ):
                n_t = 3 * KS2
                for hg in range(4):
                    if 1 <= hg < 3:
                        fetch_w2(hg + 1)
                    pas = [ps2a.tile([P, CA], dt.float32, tag=f"fa{ht}",
                                     name=f"fa{ht}") for ht in range(4)]
                    pbs = ([ps2b.tile([P, CB], dt.float32, tag=f"fb{ht}",
                                      name=f"fb{ht}")[:] for ht in range(4)]
                           if CB else None)
                    for i2 in range(KS2):
                        th, tl = w2_tiles[(hg, i2)]
                        for ht in range(4):
                            hsl = slice(ht * P, (ht + 1) * P)
                            for ti3, (wt, mt) in enumerate(
                                    ((th, hh[i2]), (th, hl[i2]), (tl, hh[i2]))):
                                ti = i2 * 3 + ti3
                                nc.tensor.matmul(
                                    pas[ht][:], wt[:, :, hsl], mt[:, :, 0:CA],
                                    start=(ti == 0), stop=(ti == n_t - 1),
                                    perf_mode=DR)
                                if CB:
                                    nc.tensor.matmul(
                                        pbs[ht], wt[:, :, hsl], mt[:, :, CA:C],
                                        start=(ti == 0), stop=(ti == n_t - 1),
                                        perf_mode=DR)
                    last = hg == 3
                    for ht in range(4):
                        hrow = hg * 4 + ht
                        # last group: split store queues so the final DMAs
                        # issue in parallel (gpsimd lacks PSUM access, so all
                        # drains stay on DVE)
                        dma_eng = nc.sync if (last and ht % 2) else nc.scalar
                        yt = ytp.tile([P, C], dt.float16, tag="yt", name="yt")
                        nc.vector.tensor_tensor(out=yt[:, 0:CA], in0=pas[ht][:],
                                                in1=g_sb[:, 0:CA], op=OP.mult)
                        if CB:
                            nc.vector.tensor_tensor(out=yt[:, CA:C], in0=pbs[ht],
                                                    in1=g_sb[:, CA:C], op=OP.mult)
                        dma_eng.dma_start(yt_d[hrow * P:(hrow + 1) * P, :], yt[:])
                    for i2 in range(KS2):
                        del w2_tiles[(hg, i2)]
            w2p.__exit__(None, None, None)

    nc.compile()
    return nc


def _get_compiled(C=545, mode="fp8"):
    key = (C, mode)
    if key not in _COMPILED:
        _COMPILED[key] = _build_fp8(C) if mode == "fp8" else _build(C)
    return _COMPILED[key]


def _route(x, gate_w, gate_b, alpha):
    """Exact host gate: returns (tok_lists, g_lists, gates_dense)."""
    lg = x.astype(np.float64) @ gate_w.astype(np.float64) + gate_b.astype(np.float64)
    m = lg.max(axis=1, keepdims=True)
    sm = np.exp(lg - m)
    sm /= sm.sum(axis=1, keepdims=True)
    top2 = np.argpartition(-lg, TOP_K - 1, axis=1)[:, :TOP_K]
    gates = np.zeros((x.shape[0], E), np.float64)
    rows = np.arange(x.shape[0])[:, None]
    gates[rows, top2] = np.take_along_axis(sm, top2, axis=1)
    gates *= alpha.astype(np.float64)[None, :]
    mask = np.zeros((x.shape[0], E), bool)
    mask[rows, top2] = True
    toks = [np.where(mask[:, e])[0] for e in range(E)]
    gs = [gates[toks[e], e].astype(np.float32) for e in range(E)]
    return toks, gs, gates.astype(np.float32)


def _split8(a, scale):
    """hi/lo fp8e4 residual pair of a*scale (ml_dtypes arrays)."""
    import ml_dtypes
    F8 = ml_dtypes.float8_e4m3
    s = a.astype(np.float32) * np.float32(scale)
    hi = s.astype(F8)
    lo = (s - hi.astype(np.float32)).astype(F8)
    return hi, lo


def _dr_layout(a, ksteps):
    """[K, N] -> [ksteps*128, 2, N] with k = ks*256 + s*128 + p."""
    K, N = a.shape
    assert K == ksteps * 256
    return np.ascontiguousarray(
        a.reshape(ksteps, 2, P, N).transpose(0, 2, 1, 3).reshape(ksteps * P, 2, N))


def _in_maps_fp16(x, toks, gs, fc1_w, fc1_b, fc2_w, C):
    in_maps = []
    for e in range(E):
        L = len(toks[e])
        xgt = np.zeros((H, C), np.float16)
        xgt[:, :L] = x[toks[e]].T.astype(np.float16)
        g = np.zeros((P, C), np.float32)
        g[:, :L] = gs[e][None, :]
        in_maps.append({
            "w1t": np.ascontiguousarray(np.asarray(fc1_w[e], np.float16)),
            "w2": np.ascontiguousarray(np.asarray(fc2_w[e], np.float16)),
            "xgt": xgt,
            "g": g,
            "b1": np.ascontiguousarray(
                np.asarray(fc1_b[e], np.float32).reshape(IT, P).T),
        })
    return in_maps


def _in_maps_fp8(x, toks, gs, fc1_w, fc1_b, fc2_w, C):
    in_maps = []
    for e in range(E):
        L = len(toks[e])
        xgt = np.zeros((H, C), np.float32)
        xgt[:, :L] = x[toks[e]].T
        xh, xl = _split8(_dr_layout(xgt, H // 256), SB)
        xhl = np.concatenate([xh, xl], axis=2)
        w1h, w1l = _split8(_dr_layout(
            np.asarray(fc1_w[e], np.float32), H // 256), SA)
        w2h, w2l = _split8(_dr_layout(
            np.asarray(fc2_w[e], np.float32), I // 256), SA)
        g = np.zeros((P, C), np.float32)
        g[:, :L] = gs[e][None, :] / np.float32(SA * SC)
        in_maps.append({
            "w1h": w1h, "w1l": w1l, "w2h": w2h, "w2l": w2l,
            "xhl": xhl, "g": g,
            "b1": np.ascontiguousarray(
                np.asarray(fc1_b[e], np.float32).reshape(IT, P).T),
        })
    return in_maps


def kernel(hidden_states, gate_w, gate_b, fc1_w, fc1_b, fc2_w, fc2_b, alpha,
           mode="fp8"):
    from concourse.bass_utils import run_bass_kernel_spmd

    x = np.ascontiguousarray(np.asarray(hidden_states, np.float32).reshape(T, H))
    toks, gs, gates = _route(x, np.asarray(gate_w, np.float32),
                             np.asarray(gate_b, np.float32),
                             np.asarray(alpha, np.float32))
    C = max(max(len(t) for t in toks), 1)
    nc = _get_compiled(C, mode)

    if mode == "fp8":
        in_maps = _in_maps_fp8(x, toks, gs, fc1_w, fc1_b, fc2_w, C)
    else:
        in_maps = _in_maps_fp16(x, toks, gs, fc1_w, fc1_b, fc2_w, C)

    res = run_bass_kernel_spmd(nc, in_maps, core_ids=list(range(E)), trace=False)

    out = np.zeros((T, H), np.float32)
    for e in range(E):
        L = len(toks[e])
        if L:
            out[toks[e]] += res.results[e]["yt"].T[:L].astype(np.float32)
    out += gates @ np.asarray(fc2_b, np.float32)
    return out.reshape(B, S_SEQ, H)
